# revision 3
# baseline (speedup 1.0000x reference)
"""GCN (DirectionalGraphConvolution) Trainium2 kernel v2, 8-core SPMD.

Math (per reference):
    deg[n]  = 1 + sum_{e: dst==n} w_e
    dinv    = deg ** -0.5
    out[d]  = relu( (sum_e norm_e * x[src_e]) @ W + bias ),
    norm_e  = dinv[src_e] * w_e * dinv[dst_e]   (self loop: dinv[d]^2)

v2 changes vs baseline:
  * norm (incl. dinv[src] and dinv[dst]) folded on the host into the
    per-edge weight array -> no on-device degree stage, no AllGather, no
    dinv column in the gather table.
  * gather table is plain x rows: bf16 [NPAD, 128] = 256B/row (or f32
    512B/row via XDT) instead of 768B augmented rows.
  * self-loops are not gathered: each dst tile gets one extra chunk whose
    rows are a contiguous dma of the tile's own x rows and whose sel is
    diag(dinv^2) via the same is_equal one-hot path.
  * aggregation matmul swapped to out[ch, dst] = g[e, ch].T @ sel[e, dst]
    so PSUM holds agg transposed -> the per-tile PE transpose disappears;
    the weight matmul consumes aggT directly.
"""

import numpy as np

import concourse.bass as bass
import concourse.bacc as bacc
import concourse.tile as tile
import concourse.mybir as mybir
from concourse import bass_utils

N = 50000
E = 800000
CIN = 128
COUT = 128
N_CORES = 8
TPC = 49                    # dst tiles per core
NP_CORE = TPC * 128         # 6272
NPAD = N_CORES * NP_CORE    # 50176
SPLIT = 32768               # int16 index limit for dma_gather

F32 = mybir.dt.float32
BF16 = mybir.dt.bfloat16
I16 = mybir.dt.int16

# ---- tuning knobs ----------------------------------------------------------
XDT = BF16                  # gather-table dtype (BF16: 256B rows, F32: 512B)
SELDT = BF16                # sel matrix dtype (must match gather dtype for PE)
TB = 1                      # dst tiles per gather batch
GBUFS = 8                   # gather buffer depth per stream
SELBUFS = 16
PS1BUFS = 4
NQ = 4                      # num swdge queues (round-robin across gathers)


def _wrap_idx16(vals):
    n = vals.shape[0]
    w16 = np.swapaxes(vals.reshape(n // 16, 16), -1, -2).astype(np.int16)
    return np.tile(w16, (8, 1))


def _preprocess(x, edge_index, edge_weight):
    src = np.asarray(edge_index[0], dtype=np.int64)
    dst = np.asarray(edge_index[1], dtype=np.int64)
    w = np.asarray(edge_weight, dtype=np.float32)

    # degree with self-loop weight 1, then dinv = deg^-0.5 (host-folded)
    deg = np.bincount(dst, weights=w.astype(np.float64), minlength=NPAD)
    deg[:N] += 1.0
    dinv = np.zeros(NPAD, np.float64)
    nz = deg > 0
    dinv[nz] = deg[nz] ** -0.5
    norm = (dinv[src] * w * dinv[dst]).astype(np.float32)
    dloop = (dinv[:NPAD] ** 2).astype(np.float32)   # self-loop coefficient

    # ---- balanced node->tile assignment (cuts per-tile chunk padding) -----
    # Snake-deal nodes in decreasing weighted-degree order across all 392
    # tiles so every (core, tile) bin sees a near-equal lo/hi edge count;
    # the output rows come back permuted and are unpermuted on the host.
    NT = N_CORES * TPC
    lo_deg = np.bincount(dst[src < SPLIT], minlength=NPAD)
    hi_deg = np.bincount(dst[src >= SPLIT], minlength=NPAD)
    order_n = np.argsort(-(2 * lo_deg[:N] + hi_deg[:N]), kind="stable")
    ids = np.concatenate([order_n, np.arange(N, NPAD)])
    perm = np.empty(NPAD, np.int64)
    slot_of_round = np.arange(128).repeat(NT)
    bin_idx = np.tile(np.arange(NT), 128).reshape(128, NT)
    bin_idx[1::2] = bin_idx[1::2, ::-1]             # snake
    perm[ids] = (bin_idx.ravel() * 128 + slot_of_round)
    invperm = np.empty(NPAD, np.int64)
    invperm[perm] = np.arange(NPAD)
    dst_p = perm[dst]

    # ---- group real edges by dst tile, sorted by src within each tile -----
    tile_g = dst_p >> 7
    order = np.lexsort((src, tile_g))
    src_s = src[order]
    dst_s = dst_p[order]
    w_s = norm[order]
    tile_s = tile_g[order]

    n_tiles_g = N_CORES * TPC
    counts = np.bincount(tile_s, minlength=n_tiles_g)
    is_hi = src_s >= SPLIT
    nlo = np.bincount(tile_s[~is_hi], minlength=n_tiles_g)
    nhi = counts - nlo

    nlo_ct = nlo.reshape(N_CORES, TPC)
    nhi_ct = nhi.reshape(N_CORES, TPC)
    K_lo = np.maximum(1, -(-nlo_ct.max(axis=0) // 128)).astype(np.int64)
    K_hi = np.maximum(1, -(-nhi_ct.max(axis=0) // 128)).astype(np.int64)
    KT = K_lo + K_hi + 1                              # +1 self-loop chunk
    coff = np.concatenate([[0], np.cumsum(KT)[:-1]])
    CC = int(KT.sum())
    lo_off = np.concatenate([[0], np.cumsum(K_lo)[:-1]])
    hi_off = np.concatenate([[0], np.cumsum(K_hi)[:-1]])
    CLO = int(K_lo.sum())
    CHI = int(K_hi.sum())

    starts = np.concatenate([[0], np.cumsum(counts)[:-1]])
    EA = src_s.shape[0]
    j_in = np.arange(EA) - np.repeat(starts, counts)
    j_hi = j_in - nlo[tile_s]
    core_e = tile_s // TPC
    t_e = tile_s % TPC
    p_e = np.where(is_hi, j_hi % 128, j_in % 128)
    k_e = np.where(is_hi, K_lo[t_e] + j_hi // 128, j_in // 128)
    col_e = coff[t_e] + k_e

    warr = np.zeros((N_CORES, 128, CC), np.float32)
    dstloc = np.zeros((N_CORES, 128, CC), np.float32)
    warr[core_e, p_e, col_e] = w_s
    dstloc[core_e, p_e, col_e] = (dst_s & 127).astype(np.float32)

    # self-loop chunk: col coff[t] + K_lo[t] + K_hi[t], sel = diag(dloop)
    slots = perm[np.arange(NPAD)]
    c_n = slots // NP_CORE
    t_n = (slots % NP_CORE) // 128
    p_n = slots % 128
    warr[c_n, p_n, coff[t_n] + K_lo[t_n] + K_hi[t_n]] = dloop
    dstloc[c_n, p_n, coff[t_n] + K_lo[t_n] + K_hi[t_n]] = p_n

    lo_m = ~is_hi
    vlo = np.zeros((N_CORES, CLO * 128), np.int64)
    vhi = np.zeros((N_CORES, CHI * 128), np.int64)
    vlo[core_e[lo_m], (lo_off[t_e[lo_m]]) * 128 + j_in[lo_m]] = src_s[lo_m]
    vhi[core_e[is_hi], (hi_off[t_e[is_hi]]) * 128 + j_hi[is_hi]] = (
        src_s[is_hi] - SPLIT)
    idxlo = np.stack([_wrap_idx16(vlo[c]) for c in range(N_CORES)])
    idxhi = np.stack([_wrap_idx16(vhi[c]) for c in range(N_CORES)])

    if XDT == BF16:
        import ml_dtypes
        x_tab = np.zeros((NPAD, CIN), ml_dtypes.bfloat16)
        x_tab[:N] = np.asarray(x).astype(ml_dtypes.bfloat16)
    else:
        x_tab = np.zeros((NPAD, CIN), np.float32)
        x_tab[:N] = np.asarray(x, dtype=np.float32)

    shapes = (tuple(K_lo.tolist()), tuple(K_hi.tolist()))
    return x_tab, warr, dstloc, idxlo, idxhi, perm, invperm, shapes


def _build(shapes, single_core=False, reps=1, bench=False):
    K_lo_t, K_hi_t = shapes
    K_lo = np.asarray(K_lo_t)
    K_hi = np.asarray(K_hi_t)
    KT = K_lo + K_hi + 1
    coff = np.concatenate([[0], np.cumsum(KT)[:-1]])
    lo_off = np.concatenate([[0], np.cumsum(K_lo)[:-1]])
    hi_off = np.concatenate([[0], np.cumsum(K_hi)[:-1]])
    CC = int(KT.sum())
    CLO = int(K_lo.sum())
    CHI = int(K_hi.sum())

    nc = bacc.Bacc("TRN2", target_bir_lowering=False, debug=False,
                   enable_asserts=False,
                   num_devices=1 if single_core else N_CORES,
                   num_swdge_queues=NQ)

    x_d = (None if bench else
           nc.dram_tensor("x", [NPAD, CIN], XDT, kind="ExternalInput").ap())
    xown_d = nc.dram_tensor("xown", [NP_CORE, CIN], XDT, kind="ExternalInput").ap()
    warr_d = nc.dram_tensor("warr", [128, CC], F32, kind="ExternalInput").ap()
    dstloc_d = nc.dram_tensor("dstloc", [128, CC], F32, kind="ExternalInput").ap()
    idxlo_d = nc.dram_tensor("idxlo", [128, CLO * 8], I16, kind="ExternalInput").ap()
    idxhi_d = nc.dram_tensor("idxhi", [128, CHI * 8], I16, kind="ExternalInput").ap()
    wt_d = nc.dram_tensor("wt", [CIN, COUT], F32, kind="ExternalInput").ap()
    bias_d = nc.dram_tensor("bias", [COUT, 1], F32, kind="ExternalInput").ap()
    outT_d = nc.dram_tensor("outT", [COUT, NP_CORE], F32, kind="ExternalOutput").ap()

    groups = [(g, min(TB, TPC - g)) for g in range(0, TPC, TB)]
    max_glo = max(int(K_lo[g:g + n].sum()) for g, n in groups)
    max_ghi = max(int(K_hi[g:g + n].sum()) for g, n in groups)

    with tile.TileContext(nc) as tc:
        with tc.tile_pool(name="const", bufs=1) as cpool, \
             tc.tile_pool(name="pers", bufs=1) as pers, \
             tc.tile_pool(name="glo", bufs=GBUFS) as glop, \
             tc.tile_pool(name="ghi", bufs=GBUFS) as ghip, \
             tc.tile_pool(name="xtile", bufs=4) as xtp, \
             tc.tile_pool(name="selbuf", bufs=SELBUFS) as selpool, \
             tc.tile_pool(name="ebuf", bufs=3) as epool, \
             tc.tile_pool(name="obuf", bufs=3) as opool, \
             tc.tile_pool(name="ps1", bufs=PS1BUFS, space="PSUM") as ps1pool, \
             tc.tile_pool(name="ps3", bufs=2, space="PSUM") as ps3pool, \
             tc.tile_pool(name="dram", bufs=1, space="DRAM") as drampool:

            if bench:
                x_bt = drampool.tile([NPAD, CIN], XDT)
                x_da = x_bt[:]
            else:
                x_da = x_d

            # ---- constants ------------------------------------------------
            iota_i = cpool.tile([128, 128], mybir.dt.int32)
            nc.gpsimd.iota(iota_i[:], pattern=[[1, 128]], base=0, channel_multiplier=0)
            iota_f = cpool.tile([128, 128], F32)
            nc.vector.tensor_copy(iota_f[:], iota_i[:])
            wt_s = cpool.tile([CIN, COUT], F32)
            nc.sync.dma_start(out=wt_s[:], in_=wt_d[:])
            bias_s = cpool.tile([COUT, 1], F32)
            nc.sync.dma_start(out=bias_s[:], in_=bias_d[:])

            # ---- persistent per-edge arrays -------------------------------
            idxlo_s = pers.tile([128, CLO * 8], I16)
            nc.sync.dma_start(out=idxlo_s[:], in_=idxlo_d[:])
            idxhi_s = pers.tile([128, CHI * 8], I16)
            nc.sync.dma_start(out=idxhi_s[:], in_=idxhi_d[:])
            warr_s = pers.tile([128, CC], F32)
            nc.sync.dma_start(out=warr_s[:], in_=warr_d[:])
            dstloc_s = pers.tile([128, CC], F32)
            nc.sync.dma_start(out=dstloc_s[:], in_=dstloc_d[:])

            # ---- main: batched gathers + per-tile one-hot matmul ----------
            import contextlib
            rep_ctx = (tc.For_i(0, reps) if reps > 1
                       else contextlib.nullcontext())
            with rep_ctx:
                for gi, (g0, gn) in enumerate(groups):
                    klo_g = int(K_lo[g0:g0 + gn].sum())
                    khi_g = int(K_hi[g0:g0 + gn].sum())
                    nlo = klo_g * 128
                    nhi = khi_g * 128
                    ilo = slice(int(lo_off[g0]) * 8, (int(lo_off[g0]) + klo_g) * 8)
                    ihi = slice(int(hi_off[g0]) * 8, (int(hi_off[g0]) + khi_g) * 8)

                    g_lo = glop.tile([128, max_glo * CIN], XDT, tag="glo")
                    nc.gpsimd.dma_gather(
                        out_ap=g_lo[:, :klo_g * CIN].rearrange(
                            "p (k e) -> p k e", e=CIN),
                        in_ap=x_da[:SPLIT, :], idxs_ap=idxlo_s[:, ilo],
                        num_idxs=nlo, num_idxs_reg=nlo, elem_size=CIN,
                        single_packet=False, queue_num=(2 * gi) % NQ)
                    g_hi = ghip.tile([128, max_ghi * CIN], XDT, tag="ghi")
                    nc.gpsimd.dma_gather(
                        out_ap=g_hi[:, :khi_g * CIN].rearrange(
                            "p (k e) -> p k e", e=CIN),
                        in_ap=x_da[SPLIT:, :], idxs_ap=idxhi_s[:, ihi],
                        num_idxs=nhi, num_idxs_reg=nhi, elem_size=CIN,
                        single_packet=False, queue_num=(2 * gi + 1) % NQ)

                    g_lo3 = g_lo[:].rearrange("p (k e) -> p k e", e=CIN)
                    g_hi3 = g_hi[:].rearrange("p (k e) -> p k e", e=CIN)

                    for ti in range(gn):
                        t = g0 + ti
                        klo_t = int(K_lo[t])
                        khi_t = int(K_hi[t])
                        kt_t = klo_t + khi_t + 1
                        c0 = int(coff[t])
                        blo = int(lo_off[t] - lo_off[g0])
                        bhi = int(hi_off[t] - hi_off[g0])

                        # own-tile x rows for the self-loop chunk
                        x_own = xtp.tile([128, CIN], XDT, tag="xown")
                        nc.sync.dma_start(
                            out=x_own[:],
                            in_=xown_d[t * 128:(t + 1) * 128, :])

                        ps1 = ps1pool.tile([128, 128], F32)
                        for k in range(kt_t):
                            sel = selpool.tile([128, 128], SELDT)
                            nc.any.tensor_scalar(
                                out=sel[:], in0=iota_f[:],
                                scalar1=dstloc_s[:, c0 + k:c0 + k + 1],
                                scalar2=warr_s[:, c0 + k:c0 + k + 1],
                                op0=mybir.AluOpType.is_equal,
                                op1=mybir.AluOpType.mult)
                            if k < klo_t:
                                lhs = g_lo3[:, blo + k, :]
                            elif k < klo_t + khi_t:
                                lhs = g_hi3[:, bhi + (k - klo_t), :]
                            else:
                                lhs = x_own[:]
                            # psT[ch, dst] += g[e, ch].T @ sel[e, dst]
                            nc.tensor.matmul(
                                out=ps1[:], lhsT=lhs, rhs=sel[:],
                                start=(k == 0), stop=(k == kt_t - 1))

                        # epilogue: aggT -> W matmul -> relu+bias -> out
                        aggT = epool.tile([128, 128], F32)
                        nc.vector.tensor_copy(aggT[:], ps1[:])
                        ps3 = ps3pool.tile([128, 128], F32)
                        nc.tensor.matmul(out=ps3[:], lhsT=wt_s[:], rhs=aggT[:],
                                         start=True, stop=True)
                        o_t = opool.tile([128, 128], F32)
                        nc.scalar.activation(
                            o_t[:], ps3[:], mybir.ActivationFunctionType.Relu,
                            bias=bias_s[:, 0:1], scale=1.0)
                        nc.sync.dma_start(
                            out=outT_d[:, t * 128:(t + 1) * 128], in_=o_t[:])

    nc.compile()
    return nc


_CACHE = {}


def _get_program(shapes):
    if shapes not in _CACHE:
        _CACHE[shapes] = _build(shapes)
    return _CACHE[shapes]


def make_in_maps(x, edge_index, edge_weight, weight, bias):
    x_tab, warr, dstloc, idxlo, idxhi, perm, invperm, shapes = _preprocess(
        x, edge_index, edge_weight)
    wt = np.ascontiguousarray(np.asarray(weight, dtype=np.float32))
    bias_col = np.ascontiguousarray(
        np.asarray(bias, dtype=np.float32).reshape(COUT, 1))
    make_in_maps.last_perm = perm
    in_maps = []
    for c in range(N_CORES):
        in_maps.append({
            "x": x_tab,
            "xown": np.ascontiguousarray(
                x_tab[invperm[c * NP_CORE:(c + 1) * NP_CORE]]),
            "warr": np.ascontiguousarray(warr[c]),
            "dstloc": np.ascontiguousarray(dstloc[c]),
            "idxlo": np.ascontiguousarray(idxlo[c]),
            "idxhi": np.ascontiguousarray(idxhi[c]),
            "wt": wt,
            "bias": bias_col,
        })
    return in_maps, shapes


def kernel(x, edge_index, edge_weight, weight, bias, _want_trace=False):
    in_maps, shapes = make_in_maps(x, edge_index, edge_weight, weight, bias)
    nc = _get_program(shapes)
    res = None
    err = None
    for _attempt in range(3):
        try:
            res = bass_utils.run_bass_kernel_spmd(
                nc, in_maps, core_ids=list(range(N_CORES)), trace=_want_trace)
            break
        except Exception as e:
            err = e
    if res is None:
        raise err
    rows = np.concatenate(
        [res.results[c]["outT"].T for c in range(N_CORES)], axis=0)
    out = rows[make_in_maps.last_perm[:N]]
    kernel.last_results = res
    return np.ascontiguousarray(out)


kernel.last_results = None


# revision 4
# speedup vs baseline: 1.0281x; 1.0281x over previous
"""GCN (DirectionalGraphConvolution) Trainium2 kernel v2, 8-core SPMD.

Math (per reference):
    deg[n]  = 1 + sum_{e: dst==n} w_e
    dinv    = deg ** -0.5
    out[d]  = relu( (sum_e norm_e * x[src_e]) @ W + bias ),
    norm_e  = dinv[src_e] * w_e * dinv[dst_e]   (self loop: dinv[d]^2)

v2 changes vs baseline:
  * norm (incl. dinv[src] and dinv[dst]) folded on the host into the
    per-edge weight array -> no on-device degree stage, no AllGather, no
    dinv column in the gather table.
  * gather table is plain x rows: bf16 [NPAD, 128] = 256B/row (or f32
    512B/row via XDT) instead of 768B augmented rows.
  * self-loops are not gathered: each dst tile has one extra chunk whose
    rows come from a persistent partition-major SBUF copy of the core's
    own (permuted) x rows (one contiguous DMA at startup) and whose sel
    is diag(dinv^2) via the same is_equal one-hot path.
  * aggregation matmul swapped to out[ch, dst] = g[e, ch].T @ sel[e, dst]
    so PSUM holds agg transposed -> the per-tile PE transpose disappears;
    the weight matmul consumes aggT directly.
"""

import numpy as np

import concourse.bass as bass
import concourse.bacc as bacc
import concourse.tile as tile
import concourse.mybir as mybir
from concourse import bass_utils

N = 50000
E = 800000
CIN = 128
COUT = 128
N_CORES = 8
TPC = 49                    # dst tiles per core
NP_CORE = TPC * 128         # 6272
NPAD = N_CORES * NP_CORE    # 50176
SPLIT = 32768               # int16 index limit for dma_gather

F32 = mybir.dt.float32
BF16 = mybir.dt.bfloat16
I16 = mybir.dt.int16

# ---- tuning knobs ----------------------------------------------------------
XDT = BF16                  # gather-table dtype (BF16: 256B rows, F32: 512B)
SELDT = BF16                # sel matrix dtype (must match gather dtype for PE)
TB = 1                      # dst tiles per gather batch
GBUFS = 8                   # gather buffer depth per stream
SELBUFS = 16
PS1BUFS = 4
NQ = 4                      # num swdge queues (round-robin across gathers)


def _wrap_idx16(vals):
    n = vals.shape[0]
    w16 = np.swapaxes(vals.reshape(n // 16, 16), -1, -2).astype(np.int16)
    return np.tile(w16, (8, 1))


def _preprocess(x, edge_index, edge_weight):
    src = np.asarray(edge_index[0], dtype=np.int64)
    dst = np.asarray(edge_index[1], dtype=np.int64)
    w = np.asarray(edge_weight, dtype=np.float32)

    # degree with self-loop weight 1, then dinv = deg^-0.5 (host-folded)
    deg = np.bincount(dst, weights=w.astype(np.float64), minlength=NPAD)
    deg[:N] += 1.0
    dinv = np.zeros(NPAD, np.float64)
    nz = deg > 0
    dinv[nz] = deg[nz] ** -0.5
    norm = (dinv[src] * w * dinv[dst]).astype(np.float32)
    dloop = (dinv[:NPAD] ** 2).astype(np.float32)   # self-loop coefficient

    # ---- balanced node->tile assignment (cuts per-tile chunk padding) -----
    # Snake-deal nodes in decreasing weighted-degree order across all 392
    # tiles so every (core, tile) bin sees a near-equal lo/hi edge count;
    # the output rows come back permuted and are unpermuted on the host.
    NT = N_CORES * TPC
    lo_deg = np.bincount(dst[src < SPLIT], minlength=NPAD)
    hi_deg = np.bincount(dst[src >= SPLIT], minlength=NPAD)
    order_n = np.argsort(-(2 * lo_deg[:N] + hi_deg[:N]), kind="stable")
    ids = np.concatenate([order_n, np.arange(N, NPAD)])
    perm = np.empty(NPAD, np.int64)
    slot_of_round = np.arange(128).repeat(NT)
    bin_idx = np.tile(np.arange(NT), 128).reshape(128, NT)
    bin_idx[1::2] = bin_idx[1::2, ::-1]             # snake
    perm[ids] = (bin_idx.ravel() * 128 + slot_of_round)
    invperm = np.empty(NPAD, np.int64)
    invperm[perm] = np.arange(NPAD)
    dst_p = perm[dst]

    # ---- group real edges by dst tile, sorted by src within each tile -----
    tile_g = dst_p >> 7
    order = np.lexsort((src, tile_g))
    src_s = src[order]
    dst_s = dst_p[order]
    w_s = norm[order]
    tile_s = tile_g[order]

    n_tiles_g = N_CORES * TPC
    counts = np.bincount(tile_s, minlength=n_tiles_g)
    is_hi = src_s >= SPLIT
    nlo = np.bincount(tile_s[~is_hi], minlength=n_tiles_g)
    nhi = counts - nlo

    nlo_ct = nlo.reshape(N_CORES, TPC)
    nhi_ct = nhi.reshape(N_CORES, TPC)
    K_lo = np.maximum(1, -(-nlo_ct.max(axis=0) // 128)).astype(np.int64)
    K_hi = np.maximum(1, -(-nhi_ct.max(axis=0) // 128)).astype(np.int64)
    KT = K_lo + K_hi + 1                              # +1 self-loop chunk
    coff = np.concatenate([[0], np.cumsum(KT)[:-1]])
    CC = int(KT.sum())
    lo_off = np.concatenate([[0], np.cumsum(K_lo)[:-1]])
    hi_off = np.concatenate([[0], np.cumsum(K_hi)[:-1]])
    CLO = int(K_lo.sum())
    CHI = int(K_hi.sum())

    starts = np.concatenate([[0], np.cumsum(counts)[:-1]])
    EA = src_s.shape[0]
    j_in = np.arange(EA) - np.repeat(starts, counts)
    j_hi = j_in - nlo[tile_s]
    core_e = tile_s // TPC
    t_e = tile_s % TPC
    p_e = np.where(is_hi, j_hi % 128, j_in % 128)
    k_e = np.where(is_hi, K_lo[t_e] + j_hi // 128, j_in // 128)
    col_e = coff[t_e] + k_e

    warr = np.zeros((N_CORES, 128, CC), np.float32)
    dstloc = np.zeros((N_CORES, 128, CC), np.float32)
    warr[core_e, p_e, col_e] = w_s
    dstloc[core_e, p_e, col_e] = (dst_s & 127).astype(np.float32)

    # self-loop chunk: col coff[t] + K_lo[t] + K_hi[t], sel = diag(dloop)
    slots = perm[np.arange(NPAD)]
    c_n = slots // NP_CORE
    t_n = (slots % NP_CORE) // 128
    p_n = slots % 128
    warr[c_n, p_n, coff[t_n] + K_lo[t_n] + K_hi[t_n]] = dloop
    dstloc[c_n, p_n, coff[t_n] + K_lo[t_n] + K_hi[t_n]] = p_n

    lo_m = ~is_hi
    vlo = np.zeros((N_CORES, CLO * 128), np.int64)
    vhi = np.zeros((N_CORES, CHI * 128), np.int64)
    vlo[core_e[lo_m], (lo_off[t_e[lo_m]]) * 128 + j_in[lo_m]] = src_s[lo_m]
    vhi[core_e[is_hi], (hi_off[t_e[is_hi]]) * 128 + j_hi[is_hi]] = (
        src_s[is_hi] - SPLIT)
    idxlo = np.stack([_wrap_idx16(vlo[c]) for c in range(N_CORES)])
    idxhi = np.stack([_wrap_idx16(vhi[c]) for c in range(N_CORES)])

    if XDT == BF16:
        import ml_dtypes
        x_tab = np.zeros((NPAD, CIN), ml_dtypes.bfloat16)
        x_tab[:N] = np.asarray(x).astype(ml_dtypes.bfloat16)
    else:
        x_tab = np.zeros((NPAD, CIN), np.float32)
        x_tab[:N] = np.asarray(x, dtype=np.float32)

    shapes = (tuple(K_lo.tolist()), tuple(K_hi.tolist()))
    return x_tab, warr, dstloc, idxlo, idxhi, perm, invperm, shapes


def _build(shapes, single_core=False, reps=1, bench=False):
    K_lo_t, K_hi_t = shapes
    K_lo = np.asarray(K_lo_t)
    K_hi = np.asarray(K_hi_t)
    KT = K_lo + K_hi + 1
    coff = np.concatenate([[0], np.cumsum(KT)[:-1]])
    lo_off = np.concatenate([[0], np.cumsum(K_lo)[:-1]])
    hi_off = np.concatenate([[0], np.cumsum(K_hi)[:-1]])
    CC = int(KT.sum())
    CLO = int(K_lo.sum())
    CHI = int(K_hi.sum())

    nc = bacc.Bacc("TRN2", target_bir_lowering=False, debug=False,
                   enable_asserts=False,
                   num_devices=1 if single_core else N_CORES,
                   num_swdge_queues=NQ)

    x_d = (None if bench else
           nc.dram_tensor("x", [NPAD, CIN], XDT, kind="ExternalInput").ap())
    xown_d = nc.dram_tensor("xown", [128, TPC * CIN], XDT, kind="ExternalInput").ap()
    warr_d = nc.dram_tensor("warr", [128, CC], F32, kind="ExternalInput").ap()
    dstloc_d = nc.dram_tensor("dstloc", [128, CC], F32, kind="ExternalInput").ap()
    idxlo_d = nc.dram_tensor("idxlo", [128, CLO * 8], I16, kind="ExternalInput").ap()
    idxhi_d = nc.dram_tensor("idxhi", [128, CHI * 8], I16, kind="ExternalInput").ap()
    wt_d = nc.dram_tensor("wt", [CIN, COUT], F32, kind="ExternalInput").ap()
    bias_d = nc.dram_tensor("bias", [COUT, 1], F32, kind="ExternalInput").ap()
    outT_d = nc.dram_tensor("outT", [COUT, NP_CORE], F32, kind="ExternalOutput").ap()

    groups = [(g, min(TB, TPC - g)) for g in range(0, TPC, TB)]
    max_glo = max(int(K_lo[g:g + n].sum()) for g, n in groups)
    max_ghi = max(int(K_hi[g:g + n].sum()) for g, n in groups)

    with tile.TileContext(nc) as tc:
        with tc.tile_pool(name="const", bufs=1) as cpool, \
             tc.tile_pool(name="pers", bufs=1) as pers, \
             tc.tile_pool(name="glo", bufs=GBUFS) as glop, \
             tc.tile_pool(name="ghi", bufs=GBUFS) as ghip, \
             tc.tile_pool(name="selbuf", bufs=SELBUFS) as selpool, \
             tc.tile_pool(name="ebuf", bufs=3) as epool, \
             tc.tile_pool(name="obuf", bufs=3) as opool, \
             tc.tile_pool(name="ps1", bufs=PS1BUFS, space="PSUM") as ps1pool, \
             tc.tile_pool(name="ps3", bufs=2, space="PSUM") as ps3pool, \
             tc.tile_pool(name="dram", bufs=1, space="DRAM") as drampool:

            if bench:
                x_bt = drampool.tile([NPAD, CIN], XDT)
                x_da = x_bt[:]
            else:
                x_da = x_d

            # ---- constants ------------------------------------------------
            iota_i = cpool.tile([128, 128], mybir.dt.int32)
            nc.gpsimd.iota(iota_i[:], pattern=[[1, 128]], base=0, channel_multiplier=0)
            iota_f = cpool.tile([128, 128], F32)
            nc.vector.tensor_copy(iota_f[:], iota_i[:])
            wt_s = cpool.tile([CIN, COUT], F32)
            nc.sync.dma_start(out=wt_s[:], in_=wt_d[:])
            bias_s = cpool.tile([COUT, 1], F32)
            nc.sync.dma_start(out=bias_s[:], in_=bias_d[:])

            # ---- persistent own-x table (self-loop chunks) ----------------
            xown_s = pers.tile([128, TPC * CIN], XDT)
            nc.sync.dma_start(out=xown_s[:], in_=xown_d[:])

            # ---- persistent per-edge arrays -------------------------------
            idxlo_s = pers.tile([128, CLO * 8], I16)
            nc.sync.dma_start(out=idxlo_s[:], in_=idxlo_d[:])
            idxhi_s = pers.tile([128, CHI * 8], I16)
            nc.sync.dma_start(out=idxhi_s[:], in_=idxhi_d[:])
            warr_s = pers.tile([128, CC], F32)
            nc.sync.dma_start(out=warr_s[:], in_=warr_d[:])
            dstloc_s = pers.tile([128, CC], F32)
            nc.sync.dma_start(out=dstloc_s[:], in_=dstloc_d[:])

            # ---- main: batched gathers + per-tile one-hot matmul ----------
            import contextlib
            rep_ctx = (tc.For_i(0, reps) if reps > 1
                       else contextlib.nullcontext())
            with rep_ctx:
                for gi, (g0, gn) in enumerate(groups):
                    klo_g = int(K_lo[g0:g0 + gn].sum())
                    khi_g = int(K_hi[g0:g0 + gn].sum())
                    nlo = klo_g * 128
                    nhi = khi_g * 128
                    ilo = slice(int(lo_off[g0]) * 8, (int(lo_off[g0]) + klo_g) * 8)
                    ihi = slice(int(hi_off[g0]) * 8, (int(hi_off[g0]) + khi_g) * 8)

                    g_lo = glop.tile([128, max_glo * CIN], XDT, tag="glo")
                    nc.gpsimd.dma_gather(
                        out_ap=g_lo[:, :klo_g * CIN].rearrange(
                            "p (k e) -> p k e", e=CIN),
                        in_ap=x_da[:SPLIT, :], idxs_ap=idxlo_s[:, ilo],
                        num_idxs=nlo, num_idxs_reg=nlo, elem_size=CIN,
                        single_packet=False, queue_num=(2 * gi) % NQ)
                    g_hi = ghip.tile([128, max_ghi * CIN], XDT, tag="ghi")
                    nc.gpsimd.dma_gather(
                        out_ap=g_hi[:, :khi_g * CIN].rearrange(
                            "p (k e) -> p k e", e=CIN),
                        in_ap=x_da[SPLIT:, :], idxs_ap=idxhi_s[:, ihi],
                        num_idxs=nhi, num_idxs_reg=nhi, elem_size=CIN,
                        single_packet=False, queue_num=(2 * gi + 1) % NQ)

                    g_lo3 = g_lo[:].rearrange("p (k e) -> p k e", e=CIN)
                    g_hi3 = g_hi[:].rearrange("p (k e) -> p k e", e=CIN)

                    for ti in range(gn):
                        t = g0 + ti
                        klo_t = int(K_lo[t])
                        khi_t = int(K_hi[t])
                        kt_t = klo_t + khi_t + 1
                        c0 = int(coff[t])
                        blo = int(lo_off[t] - lo_off[g0])
                        bhi = int(hi_off[t] - hi_off[g0])


                        ps1 = ps1pool.tile([128, 128], F32)
                        for k in range(kt_t):
                            sel = selpool.tile([128, 128], SELDT)
                            nc.any.tensor_scalar(
                                out=sel[:], in0=iota_f[:],
                                scalar1=dstloc_s[:, c0 + k:c0 + k + 1],
                                scalar2=warr_s[:, c0 + k:c0 + k + 1],
                                op0=mybir.AluOpType.is_equal,
                                op1=mybir.AluOpType.mult)
                            if k < klo_t:
                                lhs = g_lo3[:, blo + k, :]
                            elif k < klo_t + khi_t:
                                lhs = g_hi3[:, bhi + (k - klo_t), :]
                            else:
                                lhs = xown_s[:, t * CIN:(t + 1) * CIN]
                            # psT[ch, dst] += g[e, ch].T @ sel[e, dst]
                            nc.tensor.matmul(
                                out=ps1[:], lhsT=lhs, rhs=sel[:],
                                start=(k == 0), stop=(k == kt_t - 1))

                        # epilogue: aggT -> W matmul -> relu+bias -> out
                        aggT = epool.tile([128, 128], F32)
                        nc.vector.tensor_copy(aggT[:], ps1[:])
                        ps3 = ps3pool.tile([128, 128], F32)
                        nc.tensor.matmul(out=ps3[:], lhsT=wt_s[:], rhs=aggT[:],
                                         start=True, stop=True)
                        o_t = opool.tile([128, 128], F32)
                        nc.scalar.activation(
                            o_t[:], ps3[:], mybir.ActivationFunctionType.Relu,
                            bias=bias_s[:, 0:1], scale=1.0)
                        nc.sync.dma_start(
                            out=outT_d[:, t * 128:(t + 1) * 128], in_=o_t[:])

    nc.compile()
    return nc


_CACHE = {}


def _get_program(shapes):
    if shapes not in _CACHE:
        _CACHE[shapes] = _build(shapes)
    return _CACHE[shapes]


def make_in_maps(x, edge_index, edge_weight, weight, bias):
    x_tab, warr, dstloc, idxlo, idxhi, perm, invperm, shapes = _preprocess(
        x, edge_index, edge_weight)
    wt = np.ascontiguousarray(np.asarray(weight, dtype=np.float32))
    bias_col = np.ascontiguousarray(
        np.asarray(bias, dtype=np.float32).reshape(COUT, 1))
    make_in_maps.last_perm = perm
    in_maps = []
    for c in range(N_CORES):
        in_maps.append({
            "x": x_tab,
            "xown": np.ascontiguousarray(
                x_tab[invperm[c * NP_CORE:(c + 1) * NP_CORE]]
                .reshape(TPC, 128, CIN).swapaxes(0, 1).reshape(128, TPC * CIN)),
            "warr": np.ascontiguousarray(warr[c]),
            "dstloc": np.ascontiguousarray(dstloc[c]),
            "idxlo": np.ascontiguousarray(idxlo[c]),
            "idxhi": np.ascontiguousarray(idxhi[c]),
            "wt": wt,
            "bias": bias_col,
        })
    return in_maps, shapes


def kernel(x, edge_index, edge_weight, weight, bias, _want_trace=False):
    in_maps, shapes = make_in_maps(x, edge_index, edge_weight, weight, bias)
    nc = _get_program(shapes)
    res = None
    err = None
    for _attempt in range(3):
        try:
            res = bass_utils.run_bass_kernel_spmd(
                nc, in_maps, core_ids=list(range(N_CORES)), trace=_want_trace)
            break
        except Exception as e:
            err = e
    if res is None:
        raise err
    rows = np.concatenate(
        [res.results[c]["outT"].T for c in range(N_CORES)], axis=0)
    out = rows[make_in_maps.last_perm[:N]]
    kernel.last_results = res
    return np.ascontiguousarray(out)


kernel.last_results = None
